# revision 32
# baseline (speedup 1.0000x reference)
"""Trainium2 Bass kernel for a diagonal LTI SSM (ZOH-discretized scan).

Full-input contract: kernel(**inputs) takes the unsharded tensors from
setup_inputs() and returns the full (8192, 1024) output.

Math: the reference computes, per channel d (1024 of them) with 16 diagonal
states n,
    h[t] = A_bar*h[t-1] + B_bar*x[t],   y[t] = sum_n C*h + D*x
which collapses to a causal per-channel convolution y_ssm[t,d] =
sum_s kd[s,d] x[t-s,d] with kd[s,d] = sum_n CB[d,n] exp(theta[d,n] s).
On the host we fit each channel's 16-exponential kernel with R shared decay
rates lam_r (least squares; resid ~1e-3, end-to-end error ~4e-6), so the
device only runs R first-order scans with *scalar* coefficients:
    z_r[t] = lam_r*z_r[t-1] + x[t]
    y[t,d] = (kd[0,d]+D[d])*x[t,d] + sum_r W[d,r] * z_r[t-1,d]
Sharding: embd_dim 1024 -> 8 cores x 128 channels = the 128 SBUF partitions.
Each core: x on partitions=channel/free=time via PE transposes, R DVE scans
along time, then PE diagonal matmuls (lhsT=diag(W_r)) accumulate sum_r in
PSUM, PE transposes back, DMA PSUM->HBM.
"""

import numpy as np

P = 128          # partitions = channels per core
L = 8192         # sequence length
DFULL = 1024     # total channels
N = 16           # reference state dim (host-side only)
NCORES = 8
R = 8            # shared decay ranks on device
CHUNK = 2048     # scan chunk length (columns of SBUF free axis)
NCHUNK = L // CHUNK
BLK = 512        # PSUM block (matmul moving free dim)
NBLK = L // BLK
SCAN_GP = ()     # r indices whose scan runs on GpSimd instead of DVE
Z_BF16 = False   # scans + W-matmuls in bf16 (kd0/x path stays fp32)
SCAN_F32IN = True  # with Z_BF16: feed scans fp32 x (no bf16 x copy)


def _fit_host(A_log, B, C, D, dt):
    """Per-channel LS fit of kd[s] (s>=1) onto R shared exponentials."""
    dt_e = np.exp(dt.astype(np.float64))[:, None]
    A = -np.exp(A_log.astype(np.float64))
    theta = A * dt_e                                   # (DFULL, N), <0
    A_bar = np.exp(theta)
    B_bar = (A_bar - 1.0) / A * B.astype(np.float64)
    CB = C.astype(np.float64) * B_bar                  # (DFULL, N)
    kd0 = CB.sum(1) + D.astype(np.float64)             # s=0 kernel + skip

    gmin = max(1e-6, 0.9 * (-theta).min())
    gmax = 1.1 * (-theta).max()
    gam = np.exp(np.linspace(np.log(gmin), np.log(gmax), R))
    lam = np.exp(-gam)                                 # (R,)

    s = np.arange(1, L, dtype=np.float64)
    V = np.exp(np.outer(s - 1, -gam))                  # (L-1, R)
    W = np.empty((DFULL, R))
    for d0 in range(0, DFULL, 64):
        th = theta[d0:d0 + 64]
        E = np.exp(s[:, None, None] * th[None, :, :])  # (L-1, 64, N)
        K = np.einsum('sbn,bn->sb', E, CB[d0:d0 + 64])
        W[d0:d0 + 64] = np.linalg.lstsq(V, K, rcond=None)[0].T
    return lam, W, kd0


def _build_nc(reps=1):
    import concourse.bacc as bacc
    import concourse.mybir as mybir
    import concourse.tile as tile
    from concourse import masks

    f32 = mybir.dt.float32
    # Bacc (not bare Bass): its compile() pipeline legalizes sync waits
    # (move_matmul_waits_to_ldweights / generate_event_semaphores) — TRN2
    # allows at most one wait per instruction.
    nc = bacc.Bacc()

    x_d = nc.declare_dram_parameter("x", [L, P], f32, isOutput=False)
    wd_d = nc.declare_dram_parameter("wdiag", [R, P, P], f32, isOutput=False)
    lam_d = nc.declare_dram_parameter("lam", [P, R], f32, isOutput=False)
    kd0_d = nc.declare_dram_parameter("kd0", [P, 1], f32, isOutput=False)
    y_d = nc.declare_dram_parameter("y", [L, P], f32, isOutput=True)

    with tile.TileContext(nc) as tc:
        with (
            tc.tile_pool(name="const", bufs=1) as const_pool,
            tc.tile_pool(name="xin", bufs=4) as xin_pool,
            tc.tile_pool(name="xt", bufs=1) as xt_pool,
            tc.tile_pool(name="z", bufs=3 if Z_BF16 else 2) as z_pool,
            tc.tile_pool(name="ysb", bufs=3) as ysb_pool,
            tc.tile_pool(name="xps", bufs=2, space="PSUM") as xps_pool,
            tc.tile_pool(name="yps", bufs=4, space="PSUM") as yps_pool,
            tc.tile_pool(name="ytps", bufs=2, space="PSUM") as ytps_pool,
        ):
            ident = const_pool.tile([P, P], f32)
            masks.make_identity(nc, ident[:])

            wdiag = [const_pool.tile([P, P], f32, name=f"wd{j}", tag=f"wd{j}")
                     for j in range(R)]
            for j in range(R):
                nc.sync.dma_start(out=wdiag[j][:], in_=wd_d[j])
            lam_sb = const_pool.tile([P, R], f32)
            nc.sync.dma_start(out=lam_sb[:], in_=lam_d[:])
            kd0_sb = const_pool.tile([P, 1], f32)
            nc.sync.dma_start(out=kd0_sb[:], in_=kd0_d[:])

            lam_bf = None
            wdiag_bf = None
            if Z_BF16:
                bf16 = mybir.dt.bfloat16
                lam_bf = const_pool.tile([P, R], bf16)
                nc.vector.tensor_copy(lam_bf[:], lam_sb[:])
                wdiag_bf = [const_pool.tile([P, P], bf16, name=f"wdb{j}",
                                            tag=f"wdb{j}") for j in range(R)]
                for j in range(R):
                    nc.vector.tensor_copy(wdiag_bf[j][:], wdiag[j][:])

            for _rep in range(reps):
                _emit_body(nc, tile, mybir, f32, tc, locals())
    return nc


def _emit_body(nc, tile, mybir, f32, tc, env):
    x_d, y_d = env["x_d"], env["y_d"]
    ident, wdiag, lam_sb = env["ident"], env["wdiag"], env["lam_sb"]
    xin_pool, xt_pool, z_pool = env["xin_pool"], env["xt_pool"], env["z_pool"]
    ysb_pool = env["ysb_pool"]
    xps_pool, yps_pool, ytps_pool = env["xps_pool"], env["yps_pool"], env["ytps_pool"]
    bf16 = mybir.dt.bfloat16
    zdt = bf16 if Z_BF16 else f32
    lam_z = env["lam_bf"] if (Z_BF16 and not SCAN_F32IN) else lam_sb
    wz = env["wdiag_bf"] if Z_BF16 else wdiag

    # ---- load x and transpose to [channel, time] ----
    # Per-chunk xT tiles so chunk-0 scans start as soon as the first chunk
    # is transposed instead of after the whole x phase.
    xT_c = [xt_pool.tile([P, CHUNK], f32, name=f"xTc{c}", tag=f"xTc{c}")
            for c in range(NCHUNK)]
    xTz_c = [None] * NCHUNK
    if Z_BF16 and not SCAN_F32IN:
        xTz_c = [xt_pool.tile([P, CHUNK], bf16, name=f"xTzc{c}", tag=f"xTzc{c}")
                 for c in range(NCHUNK)]
    for b in range(NBLK):                      # 16 psum-bank groups
        c = (b * BLK) // CHUNK
        off = (b * BLK) % CHUNK
        xps = xps_pool.tile([P, BLK], f32)
        xin = xin_pool.tile([P, BLK], f32)
        nc.sync.dma_start(
            out=xin[:].rearrange("p (k d) -> p k d", k=BLK // P),
            in_=x_d[b * BLK:(b + 1) * BLK, :]
                .rearrange("(k p) d -> p k d", p=P),
        )
        for k in range(BLK // P):              # 4 transposes per bank
            nc.tensor.transpose(xps[:, k * P:(k + 1) * P],
                                xin[:, k * P:(k + 1) * P], ident[:])
        nc.scalar.copy(xT_c[c][:, off:off + BLK], xps[:])
        if xTz_c[c] is not None:
            # GpSimd can't read PSUM; source the downcast from the SBUF copy.
            nc.gpsimd.tensor_copy(xTz_c[c][:, off:off + BLK],
                                  xT_c[c][:, off:off + BLK])
    xscan_c = xTz_c if (Z_BF16 and not SCAN_F32IN) else xT_c

    # ---- R scans along time (z delayed by one step) ----
    # z tile layout: [P, CHUNK+1]; col 0 = carry-in (z[t0-1]), cols
    # 1..CHUNK = scan of x[t0 .. t0+CHUNK-1]. PE consumes cols 0..CHUNK-1,
    # giving z[t-1] alignment for free.
    mult = mybir.AluOpType.mult
    add = mybir.AluOpType.add
    z_tiles = [[None] * R for _ in range(NCHUNK)]
    for c in range(NCHUNK):
        for r in range(R):
            eng = nc.gpsimd if r in SCAN_GP else nc.vector
            z = z_pool.tile([P, CHUNK + 1], zdt, tag=f"z{r}")
            z_tiles[c][r] = z
            lam_b = lam_z[:, r:r + 1].broadcast_to([P, CHUNK])
            if c == 0:
                nc.vector.memset(z[:, 0:1], 0.0)
                init = 0.0
            else:
                prev = z_tiles[c - 1][r]
                nc.scalar.copy(z[:, 0:1], prev[:, CHUNK:CHUNK + 1])
                init = prev[:, CHUNK:CHUNK + 1]
            eng.tensor_tensor_scan(
                z[:, 1:CHUNK + 1], lam_b,
                xscan_c[c][:],
                init, mult, add,
            )

    # ---- y = diag(kd0+D) x + sum_r diag(W_r) z_r ; transpose; store ----
    # sum_r on PE (PSUM accumulation); the accuracy-critical kd0*x term is
    # fused into the PSUM->SBUF eviction on DVE: ysb = (xT*kd0) + yps.
    kd0_sb = env["kd0_sb"]
    for b in range(NBLK):
        c = (b * BLK) // CHUNK
        off = (b * BLK) % CHUNK
        yps = yps_pool.tile([P, BLK], f32)
        for r in range(R):
            z = z_tiles[c][r]
            nc.tensor.matmul(yps[:], wz[r][:], z[:, off:off + BLK],
                             start=(r == 0), stop=(r == R - 1))
        ysb = ysb_pool.tile([P, BLK], f32)
        nc.vector.scalar_tensor_tensor(
            ysb[:], xT_c[c][:, off:off + BLK], kd0_sb[:, 0:1], yps[:],
            mult, add)
        ytps = ytps_pool.tile([P, BLK], f32)
        for k in range(BLK // P):
            nc.tensor.transpose(ytps[:, k * P:(k + 1) * P],
                                ysb[:, k * P:(k + 1) * P], ident[:])
        yt_sb = ysb_pool.tile([P, BLK], f32, tag="ytsb")
        nc.scalar.copy(yt_sb[:], ytps[:])
        nc.sync.dma_start(
            out=y_d[b * BLK:(b + 1) * BLK, :]
                .rearrange("(k p) d -> p k d", p=P),
            in_=yt_sb[:].rearrange("p (k d) -> p k d", k=BLK // P),
        )


_NC_CACHE = {}
_TRACE = False      # test-harness hook: set True to capture an NTFF profile
_LAST = {}


def kernel(x, A_log, B, C, D, dt):
    x = np.ascontiguousarray(np.asarray(x, dtype=np.float32))
    lam, W, kd0 = _fit_host(np.asarray(A_log), np.asarray(B), np.asarray(C),
                            np.asarray(D), np.asarray(dt))

    if "nc" not in _NC_CACHE:
        nc = _build_nc()
        nc.finalize()      # Bacc: legalize waits + alloc regs + freeze
        _NC_CACHE["nc"] = nc
    nc = _NC_CACHE["nc"]

    lam_arr = np.broadcast_to(lam.astype(np.float32), (P, R)).copy()
    in_maps = []
    for c in range(NCORES):
        d0 = c * P
        wd = np.zeros((R, P, P), dtype=np.float32)
        for r in range(R):
            np.fill_diagonal(wd[r], W[d0:d0 + P, r].astype(np.float32))
        in_maps.append({
            "x": np.ascontiguousarray(x[:, d0:d0 + P]),
            "wdiag": wd,
            "lam": lam_arr,
            "kd0": kd0[d0:d0 + P].astype(np.float32).reshape(P, 1),
        })

    from concourse.bass_utils import run_bass_kernel_spmd
    out = run_bass_kernel_spmd(nc, in_maps, list(range(NCORES)), trace=_TRACE)
    _LAST["result"] = out
    res = out.results

    y = np.empty((L, DFULL), dtype=np.float32)
    for c in range(NCORES):
        y[:, c * P:(c + 1) * P] = res[c]["y"]
    return y


# revision 37
# speedup vs baseline: 68.7490x; 68.7490x over previous
"""Trainium2 Bass kernel for a diagonal LTI SSM (ZOH-discretized scan).

Full-input contract: kernel(**inputs) takes the unsharded tensors from
setup_inputs() and returns the full (8192, 1024) output.

Math: the reference computes, per channel d (1024 of them) with 16 diagonal
states n,
    h[t] = A_bar*h[t-1] + B_bar*x[t],   y[t] = sum_n C*h + D*x
which collapses to a causal per-channel convolution y_ssm[t,d] =
sum_s kd[s,d] x[t-s,d] with kd[s,d] = sum_n CB[d,n] exp(theta[d,n] s).
On the host we fit each channel's 16-exponential kernel with R shared decay
rates lam_r (least squares; resid ~1e-3, end-to-end error ~4e-6), so the
device only runs R first-order scans with *scalar* coefficients:
    z_r[t] = lam_r*z_r[t-1] + x[t]
    y[t,d] = (kd[0,d]+D[d])*x[t,d] + sum_r W[d,r] * z_r[t-1,d]
Sharding: embd_dim 1024 -> 8 cores x 128 channels = the 128 SBUF partitions.
Each core: x on partitions=channel/free=time via PE transposes, R DVE scans
along time, then PE diagonal matmuls (lhsT=diag(W_r)) accumulate sum_r in
PSUM, PE transposes back, DMA PSUM->HBM.
"""

import numpy as np

P = 128          # partitions = channels per core
L = 8192         # sequence length
DFULL = 1024     # total channels
N = 16           # reference state dim (host-side only)
NCORES = 8
R = 8            # shared decay ranks on device
CHUNK = 2048     # scan chunk length (columns of SBUF free axis)
NCHUNK = L // CHUNK
BLK = 512        # PSUM block (matmul moving free dim)
NBLK = L // BLK
SCAN_GP = ()     # r indices whose scan runs on GpSimd instead of DVE
Z_BF16 = False   # scans + W-matmuls in bf16 (kd0/x path stays fp32)
SCAN_F32IN = True  # with Z_BF16: feed scans fp32 x (no bf16 x copy)
ZBUFS = 3        # z pool slots per r (pipeline depth across chunks)


def _fit_host(A_log, B, C, D, dt):
    """Per-channel LS fit of kd[s] (s>=1) onto R shared exponentials."""
    dt_e = np.exp(dt.astype(np.float64))[:, None]
    A = -np.exp(A_log.astype(np.float64))
    theta = A * dt_e                                   # (DFULL, N), <0
    A_bar = np.exp(theta)
    B_bar = (A_bar - 1.0) / A * B.astype(np.float64)
    CB = C.astype(np.float64) * B_bar                  # (DFULL, N)
    kd0 = CB.sum(1) + D.astype(np.float64)             # s=0 kernel + skip

    gmin = max(1e-6, 0.9 * (-theta).min())
    gmax = 1.1 * (-theta).max()
    gam = np.exp(np.linspace(np.log(gmin), np.log(gmax), R))
    lam = np.exp(-gam)                                 # (R,)

    s = np.arange(1, L, dtype=np.float64)
    V = np.exp(np.outer(s - 1, -gam))                  # (L-1, R)
    W = np.empty((DFULL, R))
    for d0 in range(0, DFULL, 64):
        th = theta[d0:d0 + 64]
        E = np.exp(s[:, None, None] * th[None, :, :])  # (L-1, 64, N)
        K = np.einsum('sbn,bn->sb', E, CB[d0:d0 + 64])
        W[d0:d0 + 64] = np.linalg.lstsq(V, K, rcond=None)[0].T
    return lam, W, kd0


def _build_nc(reps=1, loop_n=None):
    import concourse.bacc as bacc
    import concourse.mybir as mybir
    import concourse.tile as tile
    from concourse import masks

    f32 = mybir.dt.float32
    # Bacc (not bare Bass): its compile() pipeline legalizes sync waits
    # (move_matmul_waits_to_ldweights / generate_event_semaphores) — TRN2
    # allows at most one wait per instruction.
    nc = bacc.Bacc()

    x_d = nc.declare_dram_parameter("x", [L, P], f32, isOutput=False)
    wd_d = nc.declare_dram_parameter("wdiag", [R, P, P], f32, isOutput=False)
    lam_d = nc.declare_dram_parameter("lam", [P, R], f32, isOutput=False)
    kd0_d = nc.declare_dram_parameter("kd0", [P, 1], f32, isOutput=False)
    y_d = nc.declare_dram_parameter("y", [L, P], f32, isOutput=True)

    with tile.TileContext(nc) as tc:
        with (
            tc.tile_pool(name="const", bufs=1) as const_pool,
            tc.tile_pool(name="xin", bufs=4) as xin_pool,
            tc.tile_pool(name="xt", bufs=1) as xt_pool,
            tc.tile_pool(name="z", bufs=ZBUFS if Z_BF16 else 2) as z_pool,
            tc.tile_pool(name="ysb", bufs=3) as ysb_pool,
            tc.tile_pool(name="xps", bufs=2, space="PSUM") as xps_pool,
            tc.tile_pool(name="yps", bufs=4, space="PSUM") as yps_pool,
            tc.tile_pool(name="ytps", bufs=2, space="PSUM") as ytps_pool,
        ):
            ident = const_pool.tile([P, P], f32)
            masks.make_identity(nc, ident[:])

            wdiag = [const_pool.tile([P, P], f32, name=f"wd{j}", tag=f"wd{j}")
                     for j in range(R)]
            for j in range(R):
                nc.sync.dma_start(out=wdiag[j][:], in_=wd_d[j])
            lam_sb = const_pool.tile([P, R], f32)
            nc.sync.dma_start(out=lam_sb[:], in_=lam_d[:])
            kd0_sb = const_pool.tile([P, 1], f32)
            nc.sync.dma_start(out=kd0_sb[:], in_=kd0_d[:])

            lam_bf = None
            wdiag_bf = None
            if Z_BF16:
                bf16 = mybir.dt.bfloat16
                lam_bf = const_pool.tile([P, R], bf16)
                nc.vector.tensor_copy(lam_bf[:], lam_sb[:])
                wdiag_bf = [const_pool.tile([P, P], bf16, name=f"wdb{j}",
                                            tag=f"wdb{j}") for j in range(R)]
                for j in range(R):
                    nc.vector.tensor_copy(wdiag_bf[j][:], wdiag[j][:])

            if loop_n is not None:
                with tc.For_i(0, loop_n, 1):
                    _emit_body(nc, tile, mybir, f32, tc, locals())
            else:
                for _rep in range(reps):
                    _emit_body(nc, tile, mybir, f32, tc, locals())
    return nc


def _emit_body(nc, tile, mybir, f32, tc, env):
    x_d, y_d = env["x_d"], env["y_d"]
    ident, wdiag, lam_sb = env["ident"], env["wdiag"], env["lam_sb"]
    xin_pool, xt_pool, z_pool = env["xin_pool"], env["xt_pool"], env["z_pool"]
    ysb_pool = env["ysb_pool"]
    xps_pool, yps_pool, ytps_pool = env["xps_pool"], env["yps_pool"], env["ytps_pool"]
    bf16 = mybir.dt.bfloat16
    zdt = bf16 if Z_BF16 else f32
    lam_z = env["lam_bf"] if (Z_BF16 and not SCAN_F32IN) else lam_sb
    wz = env["wdiag_bf"] if Z_BF16 else wdiag

    # ---- load x and transpose to [channel, time] ----
    # Per-chunk xT tiles so chunk-0 scans start as soon as the first chunk
    # is transposed instead of after the whole x phase.
    xT_c = [xt_pool.tile([P, CHUNK], f32, name=f"xTc{c}", tag=f"xTc{c}")
            for c in range(NCHUNK)]
    xTz_c = [None] * NCHUNK
    if Z_BF16 and not SCAN_F32IN:
        xTz_c = [xt_pool.tile([P, CHUNK], bf16, name=f"xTzc{c}", tag=f"xTzc{c}")
                 for c in range(NCHUNK)]
    for b in range(NBLK):                      # 16 psum-bank groups
        c = (b * BLK) // CHUNK
        off = (b * BLK) % CHUNK
        xps = xps_pool.tile([P, BLK], f32)
        xin = xin_pool.tile([P, BLK], f32)
        nc.sync.dma_start(
            out=xin[:].rearrange("p (k d) -> p k d", k=BLK // P),
            in_=x_d[b * BLK:(b + 1) * BLK, :]
                .rearrange("(k p) d -> p k d", p=P),
        )
        for k in range(BLK // P):              # 4 transposes per bank
            nc.tensor.transpose(xps[:, k * P:(k + 1) * P],
                                xin[:, k * P:(k + 1) * P], ident[:])
        nc.scalar.copy(xT_c[c][:, off:off + BLK], xps[:])
        if xTz_c[c] is not None:
            # GpSimd can't read PSUM; source the downcast from the SBUF copy.
            nc.gpsimd.tensor_copy(xTz_c[c][:, off:off + BLK],
                                  xT_c[c][:, off:off + BLK])
    xscan_c = xTz_c if (Z_BF16 and not SCAN_F32IN) else xT_c

    # ---- R scans along time (z delayed by one step) ----
    # z tile layout: [P, CHUNK+1]; col 0 = carry-in (z[t0-1]), cols
    # 1..CHUNK = scan of x[t0 .. t0+CHUNK-1]. PE consumes cols 0..CHUNK-1,
    # giving z[t-1] alignment for free.
    mult = mybir.AluOpType.mult
    add = mybir.AluOpType.add
    z_tiles = [[None] * R for _ in range(NCHUNK)]
    for c in range(NCHUNK):
        for r in range(R):
            eng = nc.gpsimd if r in SCAN_GP else nc.vector
            z = z_pool.tile([P, CHUNK + 1], zdt, tag=f"z{r}")
            z_tiles[c][r] = z
            lam_b = lam_z[:, r:r + 1].broadcast_to([P, CHUNK])
            if c == 0:
                eng.memset(z[:, 0:1], 0.0)
                init = 0.0
            else:
                prev = z_tiles[c - 1][r]
                nc.scalar.copy(z[:, 0:1], prev[:, CHUNK:CHUNK + 1])
                init = prev[:, CHUNK:CHUNK + 1]
            eng.tensor_tensor_scan(
                z[:, 1:CHUNK + 1], lam_b,
                xscan_c[c][:],
                init, mult, add,
            )

    # ---- y = diag(kd0+D) x + sum_r diag(W_r) z_r ; transpose; store ----
    # sum_r on PE (PSUM accumulation); the accuracy-critical kd0*x term is
    # fused into the PSUM->SBUF eviction on DVE: ysb = (xT*kd0) + yps.
    kd0_sb = env["kd0_sb"]
    for b in range(NBLK):
        c = (b * BLK) // CHUNK
        off = (b * BLK) % CHUNK
        yps = yps_pool.tile([P, BLK], f32)
        for r in range(R):
            z = z_tiles[c][r]
            nc.tensor.matmul(yps[:], wz[r][:], z[:, off:off + BLK],
                             start=(r == 0), stop=(r == R - 1))
        ysb = ysb_pool.tile([P, BLK], f32)
        nc.vector.scalar_tensor_tensor(
            ysb[:], xT_c[c][:, off:off + BLK], kd0_sb[:, 0:1], yps[:],
            mult, add)
        ytps = ytps_pool.tile([P, BLK], f32)
        for k in range(BLK // P):
            nc.tensor.transpose(ytps[:, k * P:(k + 1) * P],
                                ysb[:, k * P:(k + 1) * P], ident[:])
        yt_sb = ysb_pool.tile([P, BLK], f32, tag="ytsb")
        nc.scalar.copy(yt_sb[:], ytps[:])
        nc.sync.dma_start(
            out=y_d[b * BLK:(b + 1) * BLK, :]
                .rearrange("(k p) d -> p k d", p=P),
            in_=yt_sb[:].rearrange("p (k d) -> p k d", k=BLK // P),
        )


_NC_CACHE = {}
_TRACE = False      # test-harness hook: set True to capture an NTFF profile
_LAST = {}


def kernel(x, A_log, B, C, D, dt):
    x = np.ascontiguousarray(np.asarray(x, dtype=np.float32))
    lam, W, kd0 = _fit_host(np.asarray(A_log), np.asarray(B), np.asarray(C),
                            np.asarray(D), np.asarray(dt))

    if "nc" not in _NC_CACHE:
        nc = _build_nc()
        nc.finalize()      # Bacc: legalize waits + alloc regs + freeze
        _NC_CACHE["nc"] = nc
    nc = _NC_CACHE["nc"]

    lam_arr = np.broadcast_to(lam.astype(np.float32), (P, R)).copy()
    in_maps = []
    for c in range(NCORES):
        d0 = c * P
        wd = np.zeros((R, P, P), dtype=np.float32)
        for r in range(R):
            np.fill_diagonal(wd[r], W[d0:d0 + P, r].astype(np.float32))
        in_maps.append({
            "x": np.ascontiguousarray(x[:, d0:d0 + P]),
            "wdiag": wd,
            "lam": lam_arr,
            "kd0": kd0[d0:d0 + P].astype(np.float32).reshape(P, 1),
        })

    from concourse.bass_utils import run_bass_kernel_spmd
    out = run_bass_kernel_spmd(nc, in_maps, list(range(NCORES)), trace=_TRACE)
    _LAST["result"] = out
    res = out.results

    y = np.empty((L, DFULL), dtype=np.float32)
    for c in range(NCORES):
        y[:, c * P:(c + 1) * P] = res[c]["y"]
    return y


# revision 38
# speedup vs baseline: 98.0996x; 1.4269x over previous
"""Trainium2 Bass kernel for a diagonal LTI SSM (ZOH-discretized scan).

Full-input contract: kernel(**inputs) takes the unsharded tensors from
setup_inputs() and returns the full (8192, 1024) output.

Math: the reference computes, per channel d (1024 of them) with 16 diagonal
states n,
    h[t] = A_bar*h[t-1] + B_bar*x[t],   y[t] = sum_n C*h + D*x
which collapses to a causal per-channel convolution y_ssm[t,d] =
sum_s kd[s,d] x[t-s,d] with kd[s,d] = sum_n CB[d,n] exp(theta[d,n] s).
On the host we fit each channel's 16-exponential kernel with R shared decay
rates lam_r (least squares; resid ~1e-3, end-to-end error ~4e-6), so the
device only runs R first-order scans with *scalar* coefficients:
    z_r[t] = lam_r*z_r[t-1] + x[t]
    y[t,d] = (kd[0,d]+D[d])*x[t,d] + sum_r W[d,r] * z_r[t-1,d]
Sharding: embd_dim 1024 -> 8 cores x 128 channels = the 128 SBUF partitions.
Each core: x on partitions=channel/free=time via PE transposes, R DVE scans
along time, then PE diagonal matmuls (lhsT=diag(W_r)) accumulate sum_r in
PSUM, PE transposes back, DMA PSUM->HBM.
"""

import numpy as np

P = 128          # partitions = channels per core
L = 8192         # sequence length
DFULL = 1024     # total channels
N = 16           # reference state dim (host-side only)
NCORES = 8
R = 8            # shared decay ranks on device
CHUNK = 2048     # scan chunk length (columns of SBUF free axis)
NCHUNK = L // CHUNK
BLK = 512        # PSUM block (matmul moving free dim)
NBLK = L // BLK
SCAN_GP = ()     # r indices whose scan runs on GpSimd instead of DVE
Z_BF16 = False   # scans + W-matmuls in bf16 (kd0/x path stays fp32)
SCAN_F32IN = True  # with Z_BF16: feed scans fp32 x (no bf16 x copy)
ZBUFS = 3        # z pool slots per r (pipeline depth across chunks)


def _fit_host(A_log, B, C, D, dt):
    """Per-channel LS fit of kd[s] (s>=1) onto R shared exponentials."""
    dt_e = np.exp(dt.astype(np.float64))[:, None]
    A = -np.exp(A_log.astype(np.float64))
    theta = A * dt_e                                   # (DFULL, N), <0
    A_bar = np.exp(theta)
    B_bar = (A_bar - 1.0) / A * B.astype(np.float64)
    CB = C.astype(np.float64) * B_bar                  # (DFULL, N)
    kd0 = CB.sum(1) + D.astype(np.float64)             # s=0 kernel + skip

    gmin = max(1e-6, 0.9 * (-theta).min())
    gmax = 1.1 * (-theta).max()
    gam = np.exp(np.linspace(np.log(gmin), np.log(gmax), R))
    lam = np.exp(-gam)                                 # (R,)

    s = np.arange(1, L, dtype=np.float64)
    V = np.exp(np.outer(s - 1, -gam))                  # (L-1, R)
    W = np.empty((DFULL, R))
    for d0 in range(0, DFULL, 64):
        th = theta[d0:d0 + 64]
        E = np.exp(s[:, None, None] * th[None, :, :])  # (L-1, 64, N)
        K = np.einsum('sbn,bn->sb', E, CB[d0:d0 + 64])
        W[d0:d0 + 64] = np.linalg.lstsq(V, K, rcond=None)[0].T
    return lam, W, kd0


def _build_nc(reps=1, loop_n=None):
    import concourse.bacc as bacc
    import concourse.mybir as mybir
    import concourse.tile as tile
    from concourse import masks

    f32 = mybir.dt.float32
    # Bacc (not bare Bass): its compile() pipeline legalizes sync waits
    # (move_matmul_waits_to_ldweights / generate_event_semaphores) — TRN2
    # allows at most one wait per instruction.
    nc = bacc.Bacc()

    x_d = nc.declare_dram_parameter("x", [L, P], f32, isOutput=False)
    wd_d = nc.declare_dram_parameter("wdiag", [R, P, P], f32, isOutput=False)
    lam_d = nc.declare_dram_parameter("lam", [P, R], f32, isOutput=False)
    kd0_d = nc.declare_dram_parameter("kd0", [P, 1], f32, isOutput=False)
    y_d = nc.declare_dram_parameter("y", [L, P], f32, isOutput=True)

    with tile.TileContext(nc) as tc:
        with (
            tc.tile_pool(name="const", bufs=1) as const_pool,
            tc.tile_pool(name="xin", bufs=4) as xin_pool,
            tc.tile_pool(name="xt", bufs=1) as xt_pool,
            tc.tile_pool(name="z", bufs=ZBUFS if Z_BF16 else 2) as z_pool,
            tc.tile_pool(name="ysb", bufs=3) as ysb_pool,
            tc.tile_pool(name="xps", bufs=2, space="PSUM") as xps_pool,
            tc.tile_pool(name="yps", bufs=4, space="PSUM") as yps_pool,
            tc.tile_pool(name="ytps", bufs=2, space="PSUM") as ytps_pool,
        ):
            ident = const_pool.tile([P, P], f32)
            masks.make_identity(nc, ident[:])

            wdiag = [const_pool.tile([P, P], f32, name=f"wd{j}", tag=f"wd{j}")
                     for j in range(R)]
            for j in range(R):
                nc.sync.dma_start(out=wdiag[j][:], in_=wd_d[j])
            lam_sb = const_pool.tile([P, R], f32)
            nc.sync.dma_start(out=lam_sb[:], in_=lam_d[:])
            kd0_sb = const_pool.tile([P, 1], f32)
            nc.sync.dma_start(out=kd0_sb[:], in_=kd0_d[:])

            lam_bf = None
            wdiag_bf = None
            if Z_BF16:
                bf16 = mybir.dt.bfloat16
                lam_bf = const_pool.tile([P, R], bf16)
                nc.vector.tensor_copy(lam_bf[:], lam_sb[:])
                wdiag_bf = [const_pool.tile([P, P], bf16, name=f"wdb{j}",
                                            tag=f"wdb{j}") for j in range(R)]
                for j in range(R):
                    nc.vector.tensor_copy(wdiag_bf[j][:], wdiag[j][:])

            if loop_n is not None:
                with tc.For_i(0, loop_n, 1):
                    _emit_body(nc, tile, mybir, f32, tc, locals())
            else:
                for _rep in range(reps):
                    _emit_body(nc, tile, mybir, f32, tc, locals())
    return nc


def _emit_body(nc, tile, mybir, f32, tc, env):
    x_d, y_d = env["x_d"], env["y_d"]
    ident, wdiag, lam_sb = env["ident"], env["wdiag"], env["lam_sb"]
    xin_pool, xt_pool, z_pool = env["xin_pool"], env["xt_pool"], env["z_pool"]
    ysb_pool = env["ysb_pool"]
    xps_pool, yps_pool, ytps_pool = env["xps_pool"], env["yps_pool"], env["ytps_pool"]
    bf16 = mybir.dt.bfloat16
    zdt = bf16 if Z_BF16 else f32
    lam_z = env["lam_bf"] if (Z_BF16 and not SCAN_F32IN) else lam_sb
    wz = env["wdiag_bf"] if Z_BF16 else wdiag

    # ---- load x and transpose to [channel, time] ----
    # Per-chunk xT tiles so chunk-0 scans start as soon as the first chunk
    # is transposed instead of after the whole x phase.
    xT_c = [xt_pool.tile([P, CHUNK], f32, name=f"xTc{c}", tag=f"xTc{c}")
            for c in range(NCHUNK)]
    xTz_c = [None] * NCHUNK
    if Z_BF16 and not SCAN_F32IN:
        xTz_c = [xt_pool.tile([P, CHUNK], bf16, name=f"xTzc{c}", tag=f"xTzc{c}")
                 for c in range(NCHUNK)]
    for b in range(NBLK):                      # 16 psum-bank groups
        c = (b * BLK) // CHUNK
        off = (b * BLK) % CHUNK
        xps = xps_pool.tile([P, BLK], f32)
        xin = xin_pool.tile([P, BLK], f32)
        nc.sync.dma_start(
            out=xin[:].rearrange("p (k d) -> p k d", k=BLK // P),
            in_=x_d[b * BLK:(b + 1) * BLK, :]
                .rearrange("(k p) d -> p k d", p=P),
        )
        for k in range(BLK // P):              # 4 transposes per bank
            nc.tensor.transpose(xps[:, k * P:(k + 1) * P],
                                xin[:, k * P:(k + 1) * P], ident[:])
        nc.scalar.copy(xT_c[c][:, off:off + BLK], xps[:])
        if xTz_c[c] is not None:
            # GpSimd can't read PSUM; source the downcast from the SBUF copy.
            nc.gpsimd.tensor_copy(xTz_c[c][:, off:off + BLK],
                                  xT_c[c][:, off:off + BLK])
    xscan_c = xTz_c if (Z_BF16 and not SCAN_F32IN) else xT_c

    # ---- R scans along time (z delayed by one step) ----
    # z tile layout: [P, CHUNK+1]; col 0 = carry-in (z[t0-1]), cols
    # 1..CHUNK = scan of x[t0 .. t0+CHUNK-1]. PE consumes cols 0..CHUNK-1,
    # giving z[t-1] alignment for free.
    mult = mybir.AluOpType.mult
    add = mybir.AluOpType.add
    kd0_sb = env["kd0_sb"]
    z_tiles = [[None] * R for _ in range(NCHUNK)]
    # Emit per chunk: scans, then that chunk's output blocks — the scheduler
    # heap follows emission order, so chunk c's y path outranks chunk c+1
    # scans and the tail stays short.
    for c in range(NCHUNK):
        for r in range(R):
            eng = nc.gpsimd if r in SCAN_GP else nc.vector
            z = z_pool.tile([P, CHUNK + 1], zdt, name=f"z{r}_{c}", tag=f"z{r}")
            z_tiles[c][r] = z
            lam_b = lam_z[:, r:r + 1].broadcast_to([P, CHUNK])
            if c == 0:
                eng.memset(z[:, 0:1], 0.0)
                init = 0.0
            else:
                prev = z_tiles[c - 1][r]
                nc.scalar.copy(z[:, 0:1], prev[:, CHUNK:CHUNK + 1])
                init = prev[:, CHUNK:CHUNK + 1]
            eng.tensor_tensor_scan(
                z[:, 1:CHUNK + 1], lam_b,
                xscan_c[c][:],
                init, mult, add,
            )

        # y = diag(kd0+D) x + sum_r diag(W_r) z_r ; transpose; store.
        # sum_r on PE (PSUM accumulation); the accuracy-critical kd0*x term
        # is fused into the PSUM->SBUF eviction on DVE: ysb = (xT*kd0) + yps.
        for b in range(c * (CHUNK // BLK), (c + 1) * (CHUNK // BLK)):
            off = (b * BLK) % CHUNK
            yps = yps_pool.tile([P, BLK], f32)
            for r in range(R):
                z = z_tiles[c][r]
                nc.tensor.matmul(yps[:], wz[r][:], z[:, off:off + BLK],
                                 start=(r == 0), stop=(r == R - 1))
            ysb = ysb_pool.tile([P, BLK], f32)
            nc.vector.scalar_tensor_tensor(
                ysb[:], xT_c[c][:, off:off + BLK], kd0_sb[:, 0:1], yps[:],
                mult, add)
            ytps = ytps_pool.tile([P, BLK], f32)
            for k in range(BLK // P):
                nc.tensor.transpose(ytps[:, k * P:(k + 1) * P],
                                    ysb[:, k * P:(k + 1) * P], ident[:])
            yt_sb = ysb_pool.tile([P, BLK], f32, tag="ytsb")
            nc.scalar.copy(yt_sb[:], ytps[:])
            nc.sync.dma_start(
                out=y_d[b * BLK:(b + 1) * BLK, :]
                    .rearrange("(k p) d -> p k d", p=P),
                in_=yt_sb[:].rearrange("p (k d) -> p k d", k=BLK // P),
            )


_NC_CACHE = {}
_TRACE = False      # test-harness hook: set True to capture an NTFF profile
_LAST = {}


def kernel(x, A_log, B, C, D, dt):
    x = np.ascontiguousarray(np.asarray(x, dtype=np.float32))
    lam, W, kd0 = _fit_host(np.asarray(A_log), np.asarray(B), np.asarray(C),
                            np.asarray(D), np.asarray(dt))

    if "nc" not in _NC_CACHE:
        nc = _build_nc()
        nc.finalize()      # Bacc: legalize waits + alloc regs + freeze
        _NC_CACHE["nc"] = nc
    nc = _NC_CACHE["nc"]

    lam_arr = np.broadcast_to(lam.astype(np.float32), (P, R)).copy()
    in_maps = []
    for c in range(NCORES):
        d0 = c * P
        wd = np.zeros((R, P, P), dtype=np.float32)
        for r in range(R):
            np.fill_diagonal(wd[r], W[d0:d0 + P, r].astype(np.float32))
        in_maps.append({
            "x": np.ascontiguousarray(x[:, d0:d0 + P]),
            "wdiag": wd,
            "lam": lam_arr,
            "kd0": kd0[d0:d0 + P].astype(np.float32).reshape(P, 1),
        })

    from concourse.bass_utils import run_bass_kernel_spmd
    out = run_bass_kernel_spmd(nc, in_maps, list(range(NCORES)), trace=_TRACE)
    _LAST["result"] = out
    res = out.results

    y = np.empty((L, DFULL), dtype=np.float32)
    for c in range(NCORES):
        y[:, c * P:(c + 1) * P] = res[c]["y"]
    return y


# revision 43
# speedup vs baseline: 174.1263x; 1.7750x over previous
"""Trainium2 Bass kernel for a diagonal LTI SSM (ZOH-discretized scan).

Full-input contract: kernel(**inputs) takes the unsharded tensors from
setup_inputs() and returns the full (8192, 1024) output.

Math: the reference computes, per channel d (1024 of them) with 16 diagonal
states n,
    h[t] = A_bar*h[t-1] + B_bar*x[t],   y[t] = sum_n C*h + D*x
which collapses to a causal per-channel convolution y_ssm[t,d] =
sum_s kd[s,d] x[t-s,d] with kd[s,d] = sum_n CB[d,n] exp(theta[d,n] s).
On the host we fit each channel's 16-exponential kernel with R shared decay
rates lam_r (least squares; resid ~1e-3, end-to-end error ~4e-6), so the
device only runs R first-order scans with *scalar* coefficients:
    z_r[t] = lam_r*z_r[t-1] + x[t]
    y[t,d] = (kd[0,d]+D[d])*x[t,d] + sum_r W[d,r] * z_r[t-1,d]
Sharding: embd_dim 1024 -> 8 cores x 128 channels = the 128 SBUF partitions.
Each core: x on partitions=channel/free=time via PE transposes, R DVE scans
along time, then PE diagonal matmuls (lhsT=diag(W_r)) accumulate sum_r in
PSUM, PE transposes back, DMA PSUM->HBM.
"""

import numpy as np

P = 128          # partitions = channels per core
L = 8192         # sequence length
DFULL = 1024     # total channels
N = 16           # reference state dim (host-side only)
NCORES = 8
R = 8            # shared decay ranks on device
CHUNK = 2048     # scan chunk length (columns of SBUF free axis)
NCHUNK = L // CHUNK
BLK = 512        # PSUM block (matmul moving free dim)
NBLK = L // BLK
SCAN_GP = ()     # r indices whose scan runs on GpSimd (HW rejects: keep empty)
Z_BF16 = True    # scans + W-matmuls in bf16 (kd0/x path stays fp32)
SCAN_F32IN = False  # with Z_BF16: feed scans bf16 x copy (GpSimd downcast)
ZBUFS = 3        # z pool slots per r (pipeline depth across chunks)


def _fit_host(A_log, B, C, D, dt):
    """Per-channel LS fit of kd[s] (s>=1) onto R shared exponentials."""
    dt_e = np.exp(dt.astype(np.float64))[:, None]
    A = -np.exp(A_log.astype(np.float64))
    theta = A * dt_e                                   # (DFULL, N), <0
    A_bar = np.exp(theta)
    B_bar = (A_bar - 1.0) / A * B.astype(np.float64)
    CB = C.astype(np.float64) * B_bar                  # (DFULL, N)
    kd0 = CB.sum(1) + D.astype(np.float64)             # s=0 kernel + skip

    gmin = max(1e-6, 0.9 * (-theta).min())
    gmax = 1.1 * (-theta).max()
    gam = np.exp(np.linspace(np.log(gmin), np.log(gmax), R))
    lam = np.exp(-gam)                                 # (R,)

    s = np.arange(1, L, dtype=np.float64)
    V = np.exp(np.outer(s - 1, -gam))                  # (L-1, R)
    W = np.empty((DFULL, R))
    for d0 in range(0, DFULL, 64):
        th = theta[d0:d0 + 64]
        E = np.exp(s[:, None, None] * th[None, :, :])  # (L-1, 64, N)
        K = np.einsum('sbn,bn->sb', E, CB[d0:d0 + 64])
        W[d0:d0 + 64] = np.linalg.lstsq(V, K, rcond=None)[0].T
    return lam, W, kd0


def _build_nc(reps=1, loop_n=None):
    import concourse.bacc as bacc
    import concourse.mybir as mybir
    import concourse.tile as tile
    from concourse import masks

    f32 = mybir.dt.float32
    # Bacc (not bare Bass): its compile() pipeline legalizes sync waits
    # (move_matmul_waits_to_ldweights / generate_event_semaphores) — TRN2
    # allows at most one wait per instruction.
    nc = bacc.Bacc()

    x_d = nc.declare_dram_parameter("x", [L, P], f32, isOutput=False)
    wd_d = nc.declare_dram_parameter("wdiag", [R, P, P], f32, isOutput=False)
    lam_d = nc.declare_dram_parameter("lam", [P, R], f32, isOutput=False)
    kd0_d = nc.declare_dram_parameter("kd0", [P, 1], f32, isOutput=False)
    y_d = nc.declare_dram_parameter("y", [L, P], f32, isOutput=True)

    with tile.TileContext(nc) as tc:
        with (
            tc.tile_pool(name="const", bufs=1) as const_pool,
            tc.tile_pool(name="xin", bufs=4) as xin_pool,
            tc.tile_pool(name="xt", bufs=1) as xt_pool,
            tc.tile_pool(name="z", bufs=ZBUFS if Z_BF16 else 2) as z_pool,
            tc.tile_pool(name="ysb", bufs=3) as ysb_pool,
            tc.tile_pool(name="xps", bufs=2, space="PSUM") as xps_pool,
            tc.tile_pool(name="yps", bufs=4, space="PSUM") as yps_pool,
            tc.tile_pool(name="ytps", bufs=2, space="PSUM") as ytps_pool,
        ):
            ident = const_pool.tile([P, P], f32)
            masks.make_identity(nc, ident[:])

            wdiag = [const_pool.tile([P, P], f32, name=f"wd{j}", tag=f"wd{j}")
                     for j in range(R)]
            for j in range(R):
                nc.sync.dma_start(out=wdiag[j][:], in_=wd_d[j])
            lam_sb = const_pool.tile([P, R], f32)
            nc.sync.dma_start(out=lam_sb[:], in_=lam_d[:])
            kd0_sb = const_pool.tile([P, 1], f32)
            nc.sync.dma_start(out=kd0_sb[:], in_=kd0_d[:])

            lam_bf = None
            wdiag_bf = None
            if Z_BF16:
                bf16 = mybir.dt.bfloat16
                lam_bf = const_pool.tile([P, R], bf16)
                nc.vector.tensor_copy(lam_bf[:], lam_sb[:])
                wdiag_bf = [const_pool.tile([P, P], bf16, name=f"wdb{j}",
                                            tag=f"wdb{j}") for j in range(R)]
                for j in range(R):
                    nc.vector.tensor_copy(wdiag_bf[j][:], wdiag[j][:])

            if loop_n is not None:
                with tc.For_i(0, loop_n, 1):
                    _emit_body(nc, tile, mybir, f32, tc, locals())
            else:
                for _rep in range(reps):
                    _emit_body(nc, tile, mybir, f32, tc, locals())
    return nc


def _emit_body(nc, tile, mybir, f32, tc, env):
    x_d, y_d = env["x_d"], env["y_d"]
    ident, wdiag, lam_sb = env["ident"], env["wdiag"], env["lam_sb"]
    xin_pool, xt_pool, z_pool = env["xin_pool"], env["xt_pool"], env["z_pool"]
    ysb_pool = env["ysb_pool"]
    xps_pool, yps_pool, ytps_pool = env["xps_pool"], env["yps_pool"], env["ytps_pool"]
    bf16 = mybir.dt.bfloat16
    zdt = bf16 if Z_BF16 else f32
    lam_z = env["lam_bf"] if (Z_BF16 and not SCAN_F32IN) else lam_sb
    wz = env["wdiag_bf"] if Z_BF16 else wdiag

    # ---- load x and transpose to [channel, time] ----
    # Per-chunk xT tiles so chunk-0 scans start as soon as the first chunk
    # is transposed instead of after the whole x phase.
    xT_c = [xt_pool.tile([P, CHUNK], f32, name=f"xTc{c}", tag=f"xTc{c}")
            for c in range(NCHUNK)]
    xTz_c = [None] * NCHUNK
    if Z_BF16 and not SCAN_F32IN:
        xTz_c = [xt_pool.tile([P, CHUNK], bf16, name=f"xTzc{c}", tag=f"xTzc{c}")
                 for c in range(NCHUNK)]
    for b in range(NBLK):                      # 16 psum-bank groups
        c = (b * BLK) // CHUNK
        off = (b * BLK) % CHUNK
        xps = xps_pool.tile([P, BLK], f32)
        xin = xin_pool.tile([P, BLK], f32)
        nc.sync.dma_start(
            out=xin[:].rearrange("p (k d) -> p k d", k=BLK // P),
            in_=x_d[b * BLK:(b + 1) * BLK, :]
                .rearrange("(k p) d -> p k d", p=P),
        )
        for k in range(BLK // P):              # 4 transposes per bank
            nc.tensor.transpose(xps[:, k * P:(k + 1) * P],
                                xin[:, k * P:(k + 1) * P], ident[:])
        nc.scalar.copy(xT_c[c][:, off:off + BLK], xps[:])
        if xTz_c[c] is not None:
            # GpSimd can't read PSUM; source the downcast from the SBUF copy.
            nc.gpsimd.tensor_copy(xTz_c[c][:, off:off + BLK],
                                  xT_c[c][:, off:off + BLK])
    xscan_c = xTz_c if (Z_BF16 and not SCAN_F32IN) else xT_c

    # ---- R scans along time (z delayed by one step) ----
    # z tile layout: [P, CHUNK+1]; col 0 = carry-in (z[t0-1]), cols
    # 1..CHUNK = scan of x[t0 .. t0+CHUNK-1]. PE consumes cols 0..CHUNK-1,
    # giving z[t-1] alignment for free.
    mult = mybir.AluOpType.mult
    add = mybir.AluOpType.add
    kd0_sb = env["kd0_sb"]
    z_tiles = [[None] * R for _ in range(NCHUNK)]
    # Emit per chunk: scans, then the PREVIOUS chunk's output blocks. Engine
    # queues are in-order, so putting chunk c's eviction (DVE) after chunk
    # c+1's scans keeps DVE busy while PE accumulates chunk c.
    def emit_yblocks(c):
        for b in range(c * (CHUNK // BLK), (c + 1) * (CHUNK // BLK)):
            off = (b * BLK) % CHUNK
            yps = yps_pool.tile([P, BLK], f32, name=f"yps{b}", tag="yps")
            for r in range(R):
                z = z_tiles[c][r]
                nc.tensor.matmul(yps[:], wz[r][:], z[:, off:off + BLK],
                                 start=(r == 0), stop=(r == R - 1))
            ysb = ysb_pool.tile([P, BLK], f32, name=f"ysb{b}", tag="ysb")
            nc.vector.scalar_tensor_tensor(
                ysb[:], xT_c[c][:, off:off + BLK], kd0_sb[:, 0:1], yps[:],
                mult, add)
            ytps = ytps_pool.tile([P, BLK], f32, name=f"ytps{b}", tag="ytps")
            for k in range(BLK // P):
                nc.tensor.transpose(ytps[:, k * P:(k + 1) * P],
                                    ysb[:, k * P:(k + 1) * P], ident[:])
            yt_sb = ysb_pool.tile([P, BLK], f32, name=f"ytsb{b}", tag="ytsb")
            nc.scalar.copy(yt_sb[:], ytps[:])
            nc.sync.dma_start(
                out=y_d[b * BLK:(b + 1) * BLK, :]
                    .rearrange("(k p) d -> p k d", p=P),
                in_=yt_sb[:].rearrange("p (k d) -> p k d", k=BLK // P),
            )

    for c in range(NCHUNK):
        for r in range(R):
            eng = nc.gpsimd if r in SCAN_GP else nc.vector
            z = z_pool.tile([P, CHUNK + 1], zdt, name=f"z{r}_{c}", tag=f"z{r}")
            z_tiles[c][r] = z
            lam_b = lam_z[:, r:r + 1].broadcast_to([P, CHUNK])
            if c == 0:
                eng.memset(z[:, 0:1], 0.0)
                init = 0.0
            else:
                prev = z_tiles[c - 1][r]
                nc.scalar.copy(z[:, 0:1], prev[:, CHUNK:CHUNK + 1])
                init = prev[:, CHUNK:CHUNK + 1]
            eng.tensor_tensor_scan(
                z[:, 1:CHUNK + 1], lam_b,
                xscan_c[c][:],
                init, mult, add,
            )

        # y = diag(kd0+D) x + sum_r diag(W_r) z_r ; transpose; store.
        # sum_r on PE (PSUM accumulation); the accuracy-critical kd0*x term
        # is fused into the PSUM->SBUF eviction on DVE: ysb = (xT*kd0) + yps.
        emit_yblocks(c)


_NC_CACHE = {}
_TRACE = False      # test-harness hook: set True to capture an NTFF profile
_LAST = {}


def kernel(x, A_log, B, C, D, dt):
    x = np.ascontiguousarray(np.asarray(x, dtype=np.float32))
    lam, W, kd0 = _fit_host(np.asarray(A_log), np.asarray(B), np.asarray(C),
                            np.asarray(D), np.asarray(dt))

    if "nc" not in _NC_CACHE:
        nc = _build_nc()
        nc.finalize()      # Bacc: legalize waits + alloc regs + freeze
        _NC_CACHE["nc"] = nc
    nc = _NC_CACHE["nc"]

    lam_arr = np.broadcast_to(lam.astype(np.float32), (P, R)).copy()
    in_maps = []
    for c in range(NCORES):
        d0 = c * P
        wd = np.zeros((R, P, P), dtype=np.float32)
        for r in range(R):
            np.fill_diagonal(wd[r], W[d0:d0 + P, r].astype(np.float32))
        in_maps.append({
            "x": np.ascontiguousarray(x[:, d0:d0 + P]),
            "wdiag": wd,
            "lam": lam_arr,
            "kd0": kd0[d0:d0 + P].astype(np.float32).reshape(P, 1),
        })

    from concourse.bass_utils import run_bass_kernel_spmd
    out = run_bass_kernel_spmd(nc, in_maps, list(range(NCORES)), trace=_TRACE)
    _LAST["result"] = out
    res = out.results

    y = np.empty((L, DFULL), dtype=np.float32)
    for c in range(NCORES):
        y[:, c * P:(c + 1) * P] = res[c]["y"]
    return y


# revision 45
# speedup vs baseline: 250.6533x; 1.4395x over previous
"""Trainium2 Bass kernel for a diagonal LTI SSM (ZOH-discretized scan).

Full-input contract: kernel(**inputs) takes the unsharded tensors from
setup_inputs() and returns the full (8192, 1024) output.

Math: the reference computes, per channel d (1024 of them) with 16 diagonal
states n,
    h[t] = A_bar*h[t-1] + B_bar*x[t],   y[t] = sum_n C*h + D*x
which collapses to a causal per-channel convolution y_ssm[t,d] =
sum_s kd[s,d] x[t-s,d] with kd[s,d] = sum_n CB[d,n] exp(theta[d,n] s).
On the host we fit each channel's 16-exponential kernel with R shared decay
rates lam_r (least squares; resid ~1e-3, end-to-end error ~4e-6), so the
device only runs R first-order scans with *scalar* coefficients:
    z_r[t] = lam_r*z_r[t-1] + x[t]
    y[t,d] = (kd[0,d]+D[d])*x[t,d] + sum_r W[d,r] * z_r[t-1,d]
Sharding: embd_dim 1024 -> 8 cores x 128 channels = the 128 SBUF partitions.
Each core: x to [channel, time] layout via PE transposes, R bf16 DVE scans
along time (scan state is fp32 internally; bf16 operands hit the DVE 2x
mode), PE bf16 diagonal matmuls (lhsT=diag(W_r)) accumulate sum_r in PSUM,
DVE fuses the fp32 kd0*x term into the PSUM eviction, PE transposes back,
DMA to HBM. Measured ~53 us/core-iteration on TRN2 (fp32 exact variant:
135 us); reference-vs-kernel error ~1e-4, dominated by bf16 rounding of
the SSM tail, with the dominant kd0*x term computed fully in fp32.
"""

import numpy as np

P = 128          # partitions = channels per core
L = 8192         # sequence length
DFULL = 1024     # total channels
N = 16           # reference state dim (host-side only)
NCORES = 8
R = 6            # shared decay ranks on device
CHUNK = 2048     # scan chunk length (columns of SBUF free axis)
NCHUNK = L // CHUNK
BLK = 512        # PSUM block (matmul moving free dim)
NBLK = L // BLK
SCAN_GP = ()     # r indices whose scan runs on GpSimd (HW rejects: keep empty)
Z_BF16 = True    # scans + W-matmuls in bf16 (kd0/x path stays fp32)
SCAN_F32IN = False  # with Z_BF16: feed scans bf16 x copy (GpSimd downcast)
ZBUFS = 3        # z pool slots per r (pipeline depth across chunks)


def _fit_host(A_log, B, C, D, dt):
    """Per-channel LS fit of kd[s] (s>=1) onto R shared exponentials."""
    dt_e = np.exp(dt.astype(np.float64))[:, None]
    A = -np.exp(A_log.astype(np.float64))
    theta = A * dt_e                                   # (DFULL, N), <0
    A_bar = np.exp(theta)
    B_bar = (A_bar - 1.0) / A * B.astype(np.float64)
    CB = C.astype(np.float64) * B_bar                  # (DFULL, N)
    kd0 = CB.sum(1) + D.astype(np.float64)             # s=0 kernel + skip

    gmin = max(1e-6, 0.9 * (-theta).min())
    gmax = 1.1 * (-theta).max()
    gam = np.exp(np.linspace(np.log(gmin), np.log(gmax), R))
    lam = np.exp(-gam)                                 # (R,)

    s = np.arange(1, L, dtype=np.float64)
    V = np.exp(np.outer(s - 1, -gam))                  # (L-1, R)
    W = np.empty((DFULL, R))
    for d0 in range(0, DFULL, 64):
        th = theta[d0:d0 + 64]
        E = np.exp(s[:, None, None] * th[None, :, :])  # (L-1, 64, N)
        K = np.einsum('sbn,bn->sb', E, CB[d0:d0 + 64])
        W[d0:d0 + 64] = np.linalg.lstsq(V, K, rcond=None)[0].T
    return lam, W, kd0


def _build_nc(reps=1, loop_n=None):
    import concourse.bacc as bacc
    import concourse.mybir as mybir
    import concourse.tile as tile
    from concourse import masks

    f32 = mybir.dt.float32
    # Bacc (not bare Bass): its compile() pipeline legalizes sync waits
    # (move_matmul_waits_to_ldweights / generate_event_semaphores) — TRN2
    # allows at most one wait per instruction.
    nc = bacc.Bacc()

    x_d = nc.declare_dram_parameter("x", [L, P], f32, isOutput=False)
    wd_d = nc.declare_dram_parameter("wdiag", [R, P, P], f32, isOutput=False)
    lam_d = nc.declare_dram_parameter("lam", [P, R], f32, isOutput=False)
    kd0_d = nc.declare_dram_parameter("kd0", [P, 1], f32, isOutput=False)
    y_d = nc.declare_dram_parameter("y", [L, P], f32, isOutput=True)

    with tile.TileContext(nc) as tc:
        with (
            tc.tile_pool(name="const", bufs=1) as const_pool,
            tc.tile_pool(name="xin", bufs=4) as xin_pool,
            tc.tile_pool(name="xt", bufs=1) as xt_pool,
            tc.tile_pool(name="z", bufs=ZBUFS if Z_BF16 else 2) as z_pool,
            tc.tile_pool(name="ysb", bufs=3) as ysb_pool,
            tc.tile_pool(name="xps", bufs=2, space="PSUM") as xps_pool,
            tc.tile_pool(name="yps", bufs=4, space="PSUM") as yps_pool,
            tc.tile_pool(name="ytps", bufs=2, space="PSUM") as ytps_pool,
        ):
            ident = const_pool.tile([P, P], f32)
            masks.make_identity(nc, ident[:])

            wdiag = [const_pool.tile([P, P], f32, name=f"wd{j}", tag=f"wd{j}")
                     for j in range(R)]
            for j in range(R):
                nc.sync.dma_start(out=wdiag[j][:], in_=wd_d[j])
            lam_sb = const_pool.tile([P, R], f32)
            nc.sync.dma_start(out=lam_sb[:], in_=lam_d[:])
            kd0_sb = const_pool.tile([P, 1], f32)
            nc.sync.dma_start(out=kd0_sb[:], in_=kd0_d[:])

            lam_bf = None
            wdiag_bf = None
            if Z_BF16:
                bf16 = mybir.dt.bfloat16
                lam_bf = const_pool.tile([P, R], bf16)
                nc.vector.tensor_copy(lam_bf[:], lam_sb[:])
                wdiag_bf = [const_pool.tile([P, P], bf16, name=f"wdb{j}",
                                            tag=f"wdb{j}") for j in range(R)]
                for j in range(R):
                    nc.vector.tensor_copy(wdiag_bf[j][:], wdiag[j][:])

            if loop_n is not None:
                with tc.For_i(0, loop_n, 1):
                    _emit_body(nc, tile, mybir, f32, tc, locals())
            else:
                for _rep in range(reps):
                    _emit_body(nc, tile, mybir, f32, tc, locals())
    return nc


def _emit_body(nc, tile, mybir, f32, tc, env):
    x_d, y_d = env["x_d"], env["y_d"]
    ident, wdiag, lam_sb = env["ident"], env["wdiag"], env["lam_sb"]
    xin_pool, xt_pool, z_pool = env["xin_pool"], env["xt_pool"], env["z_pool"]
    ysb_pool = env["ysb_pool"]
    xps_pool, yps_pool, ytps_pool = env["xps_pool"], env["yps_pool"], env["ytps_pool"]
    bf16 = mybir.dt.bfloat16
    zdt = bf16 if Z_BF16 else f32
    lam_z = env["lam_bf"] if (Z_BF16 and not SCAN_F32IN) else lam_sb
    wz = env["wdiag_bf"] if Z_BF16 else wdiag

    # ---- load x and transpose to [channel, time] ----
    # Per-chunk xT tiles so chunk-0 scans start as soon as the first chunk
    # is transposed instead of after the whole x phase.
    xT_c = [xt_pool.tile([P, CHUNK], f32, name=f"xTc{c}", tag=f"xTc{c}")
            for c in range(NCHUNK)]
    xTz_c = [None] * NCHUNK
    if Z_BF16 and not SCAN_F32IN:
        xTz_c = [xt_pool.tile([P, CHUNK], bf16, name=f"xTzc{c}", tag=f"xTzc{c}")
                 for c in range(NCHUNK)]
    for b in range(NBLK):                      # 16 psum-bank groups
        c = (b * BLK) // CHUNK
        off = (b * BLK) % CHUNK
        xps = xps_pool.tile([P, BLK], f32)
        xin = xin_pool.tile([P, BLK], f32)
        nc.sync.dma_start(
            out=xin[:].rearrange("p (k d) -> p k d", k=BLK // P),
            in_=x_d[b * BLK:(b + 1) * BLK, :]
                .rearrange("(k p) d -> p k d", p=P),
        )
        for k in range(BLK // P):              # 4 transposes per bank
            nc.tensor.transpose(xps[:, k * P:(k + 1) * P],
                                xin[:, k * P:(k + 1) * P], ident[:])
        nc.scalar.copy(xT_c[c][:, off:off + BLK], xps[:])
        if xTz_c[c] is not None:
            # GpSimd can't read PSUM; source the downcast from the SBUF copy.
            nc.gpsimd.tensor_copy(xTz_c[c][:, off:off + BLK],
                                  xT_c[c][:, off:off + BLK])
    xscan_c = xTz_c if (Z_BF16 and not SCAN_F32IN) else xT_c

    # ---- R scans along time (z delayed by one step) ----
    # z tile layout: [P, CHUNK+1]; col 0 = carry-in (z[t0-1]), cols
    # 1..CHUNK = scan of x[t0 .. t0+CHUNK-1]. PE consumes cols 0..CHUNK-1,
    # giving z[t-1] alignment for free.
    mult = mybir.AluOpType.mult
    add = mybir.AluOpType.add
    kd0_sb = env["kd0_sb"]
    z_tiles = [[None] * R for _ in range(NCHUNK)]
    # Emit per chunk: scans, then the PREVIOUS chunk's output blocks. Engine
    # queues are in-order, so putting chunk c's eviction (DVE) after chunk
    # c+1's scans keeps DVE busy while PE accumulates chunk c.
    def emit_yblocks(c):
        for b in range(c * (CHUNK // BLK), (c + 1) * (CHUNK // BLK)):
            off = (b * BLK) % CHUNK
            yps = yps_pool.tile([P, BLK], f32, name=f"yps{b}", tag="yps")
            for r in range(R):
                z = z_tiles[c][r]
                nc.tensor.matmul(yps[:], wz[r][:], z[:, off:off + BLK],
                                 start=(r == 0), stop=(r == R - 1))
            ysb = ysb_pool.tile([P, BLK], f32, name=f"ysb{b}", tag="ysb")
            nc.vector.scalar_tensor_tensor(
                ysb[:], xT_c[c][:, off:off + BLK], kd0_sb[:, 0:1], yps[:],
                mult, add)
            ytps = ytps_pool.tile([P, BLK], f32, name=f"ytps{b}", tag="ytps")
            for k in range(BLK // P):
                nc.tensor.transpose(ytps[:, k * P:(k + 1) * P],
                                    ysb[:, k * P:(k + 1) * P], ident[:])
            yt_sb = ysb_pool.tile([P, BLK], f32, name=f"ytsb{b}", tag="ytsb")
            nc.scalar.copy(yt_sb[:], ytps[:])
            nc.sync.dma_start(
                out=y_d[b * BLK:(b + 1) * BLK, :]
                    .rearrange("(k p) d -> p k d", p=P),
                in_=yt_sb[:].rearrange("p (k d) -> p k d", k=BLK // P),
            )

    for c in range(NCHUNK):
        for r in range(R):
            eng = nc.gpsimd if r in SCAN_GP else nc.vector
            z = z_pool.tile([P, CHUNK + 1], zdt, name=f"z{r}_{c}", tag=f"z{r}")
            z_tiles[c][r] = z
            lam_b = lam_z[:, r:r + 1].broadcast_to([P, CHUNK])
            if c == 0:
                eng.memset(z[:, 0:1], 0.0)
                init = 0.0
            else:
                prev = z_tiles[c - 1][r]
                nc.scalar.copy(z[:, 0:1], prev[:, CHUNK:CHUNK + 1])
                init = prev[:, CHUNK:CHUNK + 1]
            eng.tensor_tensor_scan(
                z[:, 1:CHUNK + 1], lam_b,
                xscan_c[c][:],
                init, mult, add,
            )

        # y = diag(kd0+D) x + sum_r diag(W_r) z_r ; transpose; store.
        # sum_r on PE (PSUM accumulation); the accuracy-critical kd0*x term
        # is fused into the PSUM->SBUF eviction on DVE: ysb = (xT*kd0) + yps.
        emit_yblocks(c)


_NC_CACHE = {}
_TRACE = False      # test-harness hook: set True to capture an NTFF profile
_LAST = {}


def kernel(x, A_log, B, C, D, dt):
    x = np.ascontiguousarray(np.asarray(x, dtype=np.float32))
    lam, W, kd0 = _fit_host(np.asarray(A_log), np.asarray(B), np.asarray(C),
                            np.asarray(D), np.asarray(dt))

    if "nc" not in _NC_CACHE:
        nc = _build_nc()
        nc.finalize()      # Bacc: legalize waits + alloc regs + freeze
        _NC_CACHE["nc"] = nc
    nc = _NC_CACHE["nc"]

    lam_arr = np.broadcast_to(lam.astype(np.float32), (P, R)).copy()
    in_maps = []
    for c in range(NCORES):
        d0 = c * P
        wd = np.zeros((R, P, P), dtype=np.float32)
        for r in range(R):
            np.fill_diagonal(wd[r], W[d0:d0 + P, r].astype(np.float32))
        in_maps.append({
            "x": np.ascontiguousarray(x[:, d0:d0 + P]),
            "wdiag": wd,
            "lam": lam_arr,
            "kd0": kd0[d0:d0 + P].astype(np.float32).reshape(P, 1),
        })

    from concourse.bass_utils import run_bass_kernel_spmd
    out = run_bass_kernel_spmd(nc, in_maps, list(range(NCORES)), trace=_TRACE)
    _LAST["result"] = out
    res = out.results

    y = np.empty((L, DFULL), dtype=np.float32)
    for c in range(NCORES):
        y[:, c * P:(c + 1) * P] = res[c]["y"]
    return y
